# revision 50
# baseline (speedup 1.0000x reference)
"""CFConv-Angular (SchNet triplet message passing) on 8 Trainium2 NeuronCores.

Math (per batch b, atom a, feature f; T=512 triplets, G=F=128):
    H   = r_ij @ Wf1 + bf1                      [T, G]
    S0  = softplus(H)                           [T, G]
    Wfl = (S0 - log2) @ Wf2 + bf2               [T, F]
    y   = x @ Win                               [A, F]
    P   = y[j] * y[k] * mask                    [T, F]
    out = ssp((sum_t P * Wfl) @ Wout + bout)

Device formulation — everything TRANSPOSED so one atom's filter state is a
single [128, 512] tile (feature/g on partitions, triplet t on free):
    per atom:
      HT  = Wf1aug^T @ Raug^T        PE   [128g, 512t] — ONE matmul, ONE
                                          PSUM bank (K=26, band-packed 4
                                          atoms per 128 partitions)
      (3 atoms batched per ACT instruction; HT tiles adjacent banks)
      E   = exp(HT)                  ACT  PSUM -> SBUF f16
      S0T = ln(E + 1)                ACT  f16 (same LUT set — patched)
      QT  = Wf2^T(lhsT) @ S0T        PE   [128f, 512t], K=128, 1 bank
      outT[:, a] = sum_t QT * PT     DVE  one scalar_tensor_tensor w/ accum
                                          (PT = host-gathered yj*yk stream)
    tail (two chunks, overlapped with the loop):
      z^T = Wout^T @ (outT + PSB) + bout   (PSB = host-folded beta*sum_t P;
                                            beta = bf2 - log2*colsum(Wf2))
      final ssp + transpose run on the host during output assembly.

Pipeline: software-pipelined by group with the back stage (Q matmul +
DVE contraction) lagged LAG groups so a late Q never blocks H tiles in
the in-order PE queue; small leading/trailing groups shorten the ramp
and drain.  ACT is the bottleneck engine (~126us busy of ~145us exec):
softplus is 2 passes x 512 elem/lane/atom at a fixed 1 elem/lane/cycle,
batched 3 atoms/instruction to amortize the ~180ns access+seq overhead.

Measured dead ends (HW): DVE-offloaded polynomial softplus for a subset
of groups balances ACT/DVE busy but loses 12-20us to in-order queue
disruption; scalar_tensor_tensor has NO DVE 2x/4x perf modes; a third
output-tail chunk near the drain costs ~30us of qps contention.

The neighbor gather stays host-side: every device gather path is
descriptor- or ucode-rate-limited (SWDGE 8.4ns/desc = 1.1ms/core).

Sharding: data-parallel over the 1024 (b, a) pairs, 128 per core.
"""

import os
import sys
from contextlib import ExitStack

import numpy as np

for _p in ("/opt/trn_rl_repo", "/root/.axon_site/_ro/trn_rl_repo"):
    if os.path.isdir(_p) and _p not in sys.path:
        sys.path.append(_p)

B, A, T, NRBF, F = 2, 512, 512, 25, 128
CORES = 8
NATOMS = B * A // CORES          # 128 atoms per core
GRP = 3                          # atoms per ACT instruction batch
LOG2 = float(np.log(2.0))

# Softplus offload: every POLY_EVERY-th group computes softplus(h) ~=
# C0 + h/2 + C2 u + C4 u^2 + C6 u^3 (u = h^2, minimax fit on |h|<=2.8,
# max err 1.4e-3 incl f16 stage rounding) with ACT doing only Square and
# the Horner stages on DVE — balancing the two engines instead of
# serializing everything through ACT's exp+ln.  C0 is absorbed into the
# host-folded beta*PS term per atom.
POLY_EVERY, POLY_PHASE = 6, 3
PC0 = 0.6933721
# coefficients for u = (h/2)^2: C2*4, C4*16, C6*64
PC2R, PC4R, PC6R = 0.49582268, -0.07085296256, 0.008800261888


def _groups(natoms):
    # small leading groups prime the ACT pipeline faster (the first Exp
    # only needs one atom's H instead of three); small trailing groups
    # shorten the end-of-kernel Q->d->tail drain chain
    lead = [1, 2] if natoms >= 8 else []
    trail = [2, 1] if natoms >= 16 else []
    a = sum(lead) + sum(trail)
    sizes = list(lead)
    while a < natoms:
        n = min(GRP, natoms - a)
        sizes.append(n)
        a += n
    sizes += trail
    gs, o = [], 0
    for n in sizes:
        gs.append((o, n))
        o += n
    return gs


def _is_poly(g):
    # DVE-offloaded softplus measured NET-NEGATIVE on HW: the serial
    # 6-op chain disrupts the in-order PE/DVE queues by ~2.5us per poly
    # group vs only ~1.5us of ACT savings.  Kept for reference; disabled.
    return False


def _poly_atoms(natoms):
    atoms = set()
    for g, (a0, n) in enumerate(_groups(natoms)):
        if _is_poly(g):
            atoms.update(range(a0, a0 + n))
    return atoms

_programs = {}
_act_patch_done = False


def _patch_act_tables():
    """Make exp/ln resolve to the combined natural_log_exp_and_others LUT
    set so alternating Exp/Ln does not reload activation tables (1283ns
    per reload).  Set indices are preserved, only membership is edited."""
    global _act_patch_done
    if _act_patch_done:
        return
    import concourse.bacc as bacc_mod
    from concourse import mybir

    _orig = bacc_mod.get_activation_tables

    def patched(arch):
        AF = mybir.ActivationFunctionType
        out = {}
        for name, funcs in _orig(arch).items():
            f = set(funcs)
            if name != "natural_log_exp_and_others":
                f.discard(AF.Exp)
                f.discard(AF.Ln)
            out[name] = f
        return out

    bacc_mod.get_activation_tables = patched
    _act_patch_done = True


def _build(natoms, dbg=False):
    """Build + compile the per-core Bass program covering `natoms` atoms."""
    import concourse.bacc as bacc
    import concourse.tile as tile
    from concourse import mybir

    _patch_act_tables()

    dt = mybir.dt
    f32, f16 = dt.float32, dt.float16
    AF = mybir.ActivationFunctionType
    OP = mybir.AluOpType

    nrtblk = natoms // 4

    nc = bacc.Bacc("TRN2", debug=False)

    rt_d = nc.dram_tensor("rt", [nrtblk // 2, 128, 2, 512], f16, kind="ExternalInput")
    pt_d = nc.dram_tensor("pt", [natoms // 2, 128, 2, 512], f16, kind="ExternalInput")
    wf1r_d = nc.dram_tensor("wf1r", [128, F], f16, kind="ExternalInput")
    w2_d = nc.dram_tensor("w2", [128, F], f16, kind="ExternalInput")
    psm_d = nc.dram_tensor("psm", [128, natoms], f32, kind="ExternalInput")
    wout_d = nc.dram_tensor("wout", [128, F], f32, kind="ExternalInput")
    bout_d = nc.dram_tensor("bout", [128, 1], f32, kind="ExternalInput")
    out_d = nc.dram_tensor("out", [128, natoms], f32, kind="ExternalOutput")
    if dbg:
        htdbg_d = nc.dram_tensor("htdbg", [128, 512], f32, kind="ExternalOutput")
        s0dbg_d = nc.dram_tensor("s0dbg", [128, 512], f16, kind="ExternalOutput")
        qtdbg_d = nc.dram_tensor("qtdbg", [128, 512], f32, kind="ExternalOutput")
        otdbg_d = nc.dram_tensor("otdbg", [128, natoms], f32, kind="ExternalOutput")

    groups = _groups(natoms)

    with tile.TileContext(nc) as tc, ExitStack() as ctx:
        const = ctx.enter_context(tc.tile_pool(name="const", bufs=1))
        rt_pool = ctx.enter_context(tc.tile_pool(name="rt", bufs=5))
        pt_pool = ctx.enter_context(tc.tile_pool(name="pt", bufs=8))
        e_pool = ctx.enter_context(tc.tile_pool(name="e", bufs=2))
        s0_pool = ctx.enter_context(tc.tile_pool(name="s0", bufs=5))
        d_pool = ctx.enter_context(tc.tile_pool(name="d", bufs=2))
        misc = ctx.enter_context(tc.tile_pool(name="misc", bufs=1))
        h_ps = ctx.enter_context(tc.tile_pool(name="hps", bufs=2, space="PSUM"))
        q_ps = ctx.enter_context(tc.tile_pool(name="qps", bufs=2, space="PSUM"))

        # ---- constants; first stream tiles are DMAed before the cold
        # constants so the pipeline warms up immediately
        wf1r = const.tile([128, F], f16)
        nc.sync.dma_start(wf1r[:], wf1r_d.ap())
        # NOTE: issuing rt DMAs from the Activation HWDGE queue measured
        # 184us (vs 145us on sync): descriptor generation runs on the
        # issuing engine's sequencer and stalls activations behind it.
        # pair 0 is split so atom 0's band (rows 0:32) lands first and
        # H(0) -> Exp(0) can start before the bulk of the pair arrives
        rt_tiles = {}
        for rb2 in range(min(4, nrtblk // 2)):
            rtt = rt_pool.tile([128, 2, 512], f16, name="rt")
            if rb2 == 0:
                nc.sync.dma_start(rtt[0:32, 0, :], rt_d.ap()[0][0:32, 0, :])
                nc.sync.dma_start(rtt[32:128, 0, :], rt_d.ap()[0][32:128, 0, :])
                nc.sync.dma_start(rtt[:, 1, :], rt_d.ap()[0][:, 1, :])
            else:
                nc.sync.dma_start(rtt[:], rt_d.ap()[rb2])
            rt_tiles[rb2] = rtt
        w2 = const.tile([128, F], f16)
        nc.sync.dma_start(w2[:], w2_d.ap())
        wout = const.tile([128, F], f32)
        nc.sync.dma_start(wout[:], wout_d.ap())
        bout = const.tile([128, 1], f32)
        nc.sync.dma_start(bout[:], bout_d.ap())
        ones_f32 = const.tile([128, 1], f32)
        nc.vector.memset(ones_f32[:], 1.0)
        warm = const.tile([128, 1], f32)
        nc.scalar.activation(warm[:], ones_f32[:], AF.Exp)

        outT = misc.tile([128, natoms], f32)
        psmat = const.tile([128, natoms], f32)
        nc.sync.dma_start(psmat[:], psm_d.ap())

        pt_tiles = {}

        def fetch(a):
            """Ensure the rt block-pair for atom a is DMAed (pt is fetched
            separately, LAG groups before its d-op, so the sync FIFO always
            prioritizes rt — H tiles gate ACT, pt does not)."""
            rb2 = a // 8
            if rb2 not in rt_tiles:
                rt8 = rt_pool.tile([128, 2, 512], f16, name="rt")
                nc.sync.dma_start(rt8[:], rt_d.ap()[rb2])
                rt_tiles[rb2] = rt8

        def fetch_pt(g):
            a0, n = groups[g]
            for a in range(a0, a0 + n):
                ab = a // 2
                if ab not in pt_tiles:
                    ptile = pt_pool.tile([128, 2, 512], f16, name="pt")
                    nc.sync.dma_start(ptile[:], pt_d.ap()[ab])
                    pt_tiles[ab] = ptile

        def emit_H(g):
            """Fetch DMAs + H matmuls for group g (PE work, emitted early
            so H never sits behind the previous group's Q matmuls in the
            in-order PE queue). Returns the PSUM tile."""
            a0, n = groups[g]
            for s in range(n):
                fetch(a0 + s)
            hps = h_ps.tile([128, n * 512], f32)
            for s in range(n):
                a = a0 + s
                i = a % 4
                nc.tensor.matmul(
                    hps[:, s * 512 : (s + 1) * 512],
                    lhsT=wf1r[32 * i : 32 * i + NRBF + 1, :],
                    rhs=rt_tiles[a // 8][32 * i : 32 * i + NRBF + 1, (a // 4) % 2, :],
                    start=True,
                    stop=True,
                    tile_position=(32 * i, 0),
                )
            if dbg and a0 == 0:
                nc.sync.dma_start(htdbg_d.ap(), hps[:, 0:512])
            return hps

        # Ln batching: Exp is PSUM-limited to GRP atoms/instruction, but
        # Ln reads SBUF, so QUAD groups' Exps can share one e4 tile with
        # a single Ln.  Measured on HW: QUAD=4/LAG=6 -> 153.2us, QUAD=2/
        # LAG=4 -> 169.3us vs QUAD=1 -> 144.9us: the multi-group Ln's
        # latency bursts the Q+d traffic beyond what the 2-bank qps
        # ping-pong absorbs.  QUAD=1 (per-group Ln) is the optimum.
        QUAD = 1
        quad_info = {}
        quad_atoms = {}
        for g0 in range(0, len(groups), QUAD):
            q = g0 // QUAD
            gs = list(range(g0, min(g0 + QUAD, len(groups))))
            off = 0
            for gi in gs:
                quad_info[gi] = (q, off, gi == gs[-1])
                off += groups[gi][1]
            quad_atoms[q] = off
        quad_state = {}

        def emit_act(g, hps):
            """Exp for group g into its quad's shared e4 tile; on the
            quad's last group, one Ln over the whole quad -> s4."""
            a0, n = groups[g]
            q, off, last = quad_info[g]
            if q not in quad_state:
                qn = quad_atoms[q]
                e4 = e_pool.tile([128, qn * 512], f16, name="e4")
                s4 = s0_pool.tile([128, qn * 512], f16, name="s4")
                quad_state[q] = {"e4": e4, "s4": s4}
            st = quad_state[q]
            nc.scalar.activation(
                st["e4"][:, off * 512 : (off + n) * 512], hps[:], AF.Exp
            )
            if last:
                nc.scalar.activation(
                    st["s4"][:], st["e4"][:], AF.Ln, bias=ones_f32[:]
                )
                if dbg and q == 0:
                    nc.sync.dma_start(s0dbg_d.ap(), st["s4"][:, 0:512])
            return (st, off)

        def emit_back(g, pend_g):
            """Per-atom QT matmul + DVE contraction with PT for group g."""
            st, off = pend_g
            s4 = st["s4"]
            a0, n = groups[g]
            for s in range(n):
                a = a0 + s
                qps = q_ps.tile([128, 512], f32, name="qps", tag="q")
                nc.tensor.matmul(
                    qps[:],
                    lhsT=w2[:],
                    rhs=s4[:, (off + s) * 512 : (off + s + 1) * 512],
                    start=True,
                    stop=True,
                )
                if dbg and a == 0:
                    nc.sync.dma_start(qtdbg_d.ap(), qps[:])
                d_t = d_pool.tile([128, 512], f16)
                nc.vector.scalar_tensor_tensor(
                    d_t[:],
                    qps[:],
                    1.0,
                    pt_tiles[a // 2][:, a % 2, :],
                    op0=OP.mult,
                    op1=OP.mult,
                    accum_out=outT[:, a : a + 1],
                )
                if a % 2 == 1:
                    del pt_tiles[a // 2]

        outT2 = misc.tile([128, natoms], f32)

        def emit_tail(h0, h1):
            # out z^T[o, h0:h1] = Wout^T @ (outT + beta*PS) + bout; the
            # final shifted-softplus + transpose run on the host during
            # output assembly (saves 2 ACT passes + a PE transpose and
            # shortens the end-of-kernel dependency chain).
            n = h1 - h0
            nc.vector.tensor_add(outT2[:, h0:h1], psmat[:, h0:h1], outT[:, h0:h1])
            zo_ps = q_ps.tile([128, n], f32, name="zo_ps", tag="q")
            nc.tensor.matmul(
                zo_ps[:], lhsT=wout[:], rhs=outT2[:, h0:h1], start=True, stop=True
            )
            zf = misc.tile([128, n], f32, name=f"zf{h0}")
            nc.scalar.activation(zf[:], zo_ps[:], AF.Identity, bias=bout[:])
            nc.sync.dma_start(out_d.ap()[:, h0:h1], zf[:])

        # software pipeline, back-stage lagged by LAG groups: Q(g) reaches
        # the in-order PE queue only after H(g+LAG), so a Q blocked on a
        # late s0 cannot starve ACT of H tiles.  The output tail runs in
        # three chunks so only a 16-column chain remains after the loop.
        LAG = 2   # Q(g) emitted only after H(g+LAG): a late Q never blocks H in the in-order PE queue
        checkpoints = [(natoms // 2), natoms] if natoms >= 64 else [natoms]
        pend = {}
        tail_at = 0

        def run_back(g):
            nonlocal tail_at
            emit_back(g, pend.pop(g))
            done = groups[g][0] + groups[g][1]
            while checkpoints and done >= checkpoints[0]:
                cp = checkpoints.pop(0)
                emit_tail(tail_at, cp)
                tail_at = cp

        for g in range(len(groups)):
            hps_g = emit_H(g)
            fetch_pt(g)
            if g - LAG >= 0:
                run_back(g - LAG)
            pend[g] = emit_act(g, hps_g)
        for g in range(len(groups) - LAG, len(groups)):
            run_back(g)
        if tail_at < natoms:
            emit_tail(tail_at, natoms)
        if dbg:
            nc.sync.dma_start(otdbg_d.ap(), outT[:])

    nc.compile()
    return nc


def prep_inputs(inputs, natoms=NATOMS):
    """Full problem inputs -> list of 8 per-core input maps.

    Host-side prep: y = x @ Win (tiny projection), neighbor-gathered,
    mask-folded and multiplied P = yj*yk stream transposed to [a, f, t]
    f16, its row-sums PS, r_ij transposed into band layout, beta = bf2 -
    log2*colsum(Wf2).
    """
    x = np.asarray(inputs["x"], np.float32)
    r_ij = np.asarray(inputs["r_ij"], np.float32)
    mask = np.asarray(inputs["pairwise_mask"], np.float32)
    Wf1 = np.asarray(inputs["Wf1"], np.float32)
    bf1 = np.asarray(inputs["bf1"], np.float32)
    Wf2 = np.asarray(inputs["Wf2"], np.float32)
    bf2 = np.asarray(inputs["bf2"], np.float32)
    Win = np.asarray(inputs["Win"], np.float32)
    Wout = np.asarray(inputs["Wout"], np.float32)
    bout = np.asarray(inputs["bout"], np.float32)
    nj = np.asarray(inputs["neighbors_j"])
    nk = np.asarray(inputs["neighbors_k"])

    nrtblk = natoms // 4

    wf1aug = np.vstack([Wf1, bf1[None, :]]).astype(np.float16)  # [26, F]
    wf1r = np.zeros((128, F), np.float16)
    for i in range(4):
        wf1r[32 * i : 32 * i + NRBF + 1] = wf1aug

    colsum = Wf2.sum(axis=0)
    beta_act = (bf2 - LOG2 * colsum).astype(np.float32)          # [F]
    beta_poly = (bf2 + (PC0 - LOG2) * colsum).astype(np.float32)
    poly_atoms = _poly_atoms(natoms)
    w2 = Wf2.astype(np.float16)                   # lhsT [g, f] directly
    boutc = np.ascontiguousarray(bout.reshape(F, 1)).astype(np.float32)

    y = x @ Win                                   # [B, A, F] host projection
    mask_is_ones = bool(np.all(mask == 1.0))

    in_maps = []
    for k in range(CORES):
        b = k // (CORES // B)
        a0 = (k % (CORES // B)) * NATOMS

        # rt band layout: rt[rb, 32i+r, t] = r_ij[b, a0+4rb+i, t, r], bias
        # row of ones at 32i+NRBF
        r4 = r_ij[b, a0 : a0 + natoms]            # [na, T, NRBF]
        rr = r4.reshape(nrtblk, 4, T, NRBF).transpose(0, 1, 3, 2)  # [rb,i,r,t]
        rt = np.zeros((nrtblk, 128, 512), np.float16)
        for i in range(4):
            rt[:, 32 * i : 32 * i + NRBF] = rr[:, i]
            rt[:, 32 * i + NRBF] = 1.0
        rt = np.ascontiguousarray(
            rt.reshape(nrtblk // 2, 2, 128, 512).transpose(0, 2, 1, 3)
        )

        # P stream: pt[a, f, t] = (yj*yk*mask)[a, t, f]
        yj = y[b][nj[b, a0 : a0 + natoms]]        # [na, T, F]
        yk = y[b][nk[b, a0 : a0 + natoms]]
        if not mask_is_ones:
            yj = yj * mask[b, a0 : a0 + natoms, :, None]
        P = yj * yk                               # [na, T, F] f32
        ps = P.sum(axis=1)                        # [na, F]
        betas = np.stack(
            [beta_poly if a in poly_atoms else beta_act for a in range(natoms)]
        )                                         # [na, F]
        psm = np.ascontiguousarray((ps * betas).T).astype(np.float32)  # [F, na]
        pt = P.transpose(0, 2, 1).astype(np.float16)
        pt = np.ascontiguousarray(
            pt.reshape(natoms // 2, 2, 128, 512).transpose(0, 2, 1, 3)
        )

        m = {
            "rt": np.ascontiguousarray(rt),
            "pt": pt,
            "wf1r": wf1r,
            "w2": w2,
            "psm": psm,
            "wout": Wout.astype(np.float32),
            "bout": boutc,
        }
        in_maps.append(m)
    return in_maps


def get_program(natoms=NATOMS, dbg=False):
    key = (natoms, dbg)
    if key not in _programs:
        _programs[key] = _build(natoms, dbg)
    return _programs[key]


def assemble_output(results, natoms=NATOMS):
    """Per-core z^T arrays -> ssp -> full [B, A, F] float32."""
    out = np.zeros((B, A, F), np.float32)
    for k in range(CORES):
        b = k // (CORES // B)
        a0 = (k % (CORES // B)) * NATOMS
        zt = np.asarray(results[k]["out"], np.float32)      # [F, na]
        out[b, a0 : a0 + natoms] = (np.logaddexp(0.0, zt) - LOG2).T
    return out


def kernel(**inputs) -> np.ndarray:
    from concourse import bass_utils

    nc = get_program(NATOMS)
    in_maps = prep_inputs(inputs, NATOMS)
    res = bass_utils.run_bass_kernel_spmd(nc, in_maps, core_ids=list(range(CORES)))
    return assemble_output(res.results)


if __name__ == "__main__":
    pass
